# revision 3
# baseline (speedup 1.0000x reference)
"""Redesigned PLIF/conv kernel for TRN2.

Per scan step tau (55 steps = 5 warmup + 50 counted), all engines pipelined:
  - PE: 10 matmuls (one per scan block) compute u = MW^T @ x_col into PSUM
        (f16 inputs, fp32 accumulate), 5+5 blocks across two banks.
  - ACT: one batched PSUM->SBUF copy converts u to f16 (ut double buffer).
  - DVE (f16, 2x/4x perf modes): vp = vhat + ut; amv = (vp<1)*a;
        vhat = vp*amv  -- PLIF charge/fire/reset, state carried in vhat.
  - DMA out (gpsimd ring): each counted vp tile streams to DRAM;
        host thresholds vp >= 1 and sums spike counts (exact same boundary
        as the kernel's reset decision).

x is streamed in "first-need" permuted column order so the scan starts
within ~2us of kernel start; one zero column at stream position 0 feeds
block 0's warmup.
"""
import sys

sys.path.insert(0, "/opt/trn_rl_repo")

import contextlib

import numpy as np

import concourse.bass as bass
import concourse.mybir as mybir
from concourse.bass_utils import run_bass_kernel_spmd

# ---- problem constants ----------------------------------------------------
N_CORES = 8
N, C, T = 1024, 80, 500
Cp1 = C + 1                  # 81
NS = N // N_CORES            # 128 samples per core
NB, B, W = 10, 50, 5         # scan blocks, counted steps per block, warmup
STEPS = W + B                # 55
FD = NB * Cp1                # 810 free-dim elements per scan tile
HB = FD // 2                 # 405 per psum bank piece
NCOL = T + 1                 # stream columns incl. leading zero column
EPS = 1e-5
V_TH = 1.0

# x stream DMA chunk ends (in columns): small first chunk so step 0's
# matmuls (needing 1 + 9 columns) start as soon as possible
_ends = [10]
while _ends[-1] < NCOL:
    _ends.append(min(NCOL, _ends[-1] + 25))
CHUNK_ENDS = _ends
N_CHUNK = len(CHUNK_ENDS)    # 21
SPB = 10                     # vp-tile ring depth; one output DMA per tile

_PROGRAM_CACHE = {}


def _first_need_order():
    """Column stream order: position 0 is the zero column; real columns
    sorted by the first scan step that consumes them."""
    t = np.arange(T)
    c = t % B
    b = t // B
    tau_first = np.where((c >= B - W) & (b + 1 <= NB - 1), c - (B - W), W + c)
    order = np.argsort(tau_first, kind="stable")  # t values in stream order
    pos = np.empty(T, np.int64)
    pos[order] = np.arange(T) + 1  # +1 for zero column at position 0
    return order, pos


T_ORDER, T_POS = _first_need_order()


def _dma_gate_chunk(tau):
    """Number of x DMA chunks needed before step tau's matmuls."""
    if tau < W:
        p = 1 + (NB - 1) * (tau + 1)
    elif tau < B:
        p = 1 + (NB - 1) * W + NB * (tau - W + 1)
    else:
        p = 1 + (NB - 1) * W + NB * (B - W) + (tau - B + 1)
    p = min(p, NCOL)
    for k, e in enumerate(CHUNK_ENDS):
        if e >= p:
            return k + 1
    return N_CHUNK


def _build_program(a_val: float, paranoid: bool = False):
    """paranoid=True threads every same-engine DVE dependency through a real
    semaphore so CoreSim's race detector (which doesn't credit same-engine
    program order) can verify the cross-engine sync. Data-identical."""
    f32 = mybir.dt.float32
    f16 = mybir.dt.float16
    nc = bass.Bass()
    x_in = nc.dram_tensor("x", [Cp1, NCOL * NS], f16, kind="ExternalInput")
    mw_in = nc.dram_tensor("mw", [Cp1, Cp1], f16, kind="ExternalInput")
    sp_out = nc.dram_tensor("sp", [NS, B * FD], f16, kind="ExternalOutput")

    with contextlib.ExitStack() as ctx:
        def sem(name):
            return ctx.enter_context(nc.semaphore(name))

        def sb(name, shape, dtype):
            return ctx.enter_context(nc.sbuf_tensor(name, shape, dtype))

        mw_sem = sem("mw_sem")
        # one sem per DMA so the race detector sees unambiguous counts
        xs_sem = [sem(f"xs{k}") for k in range(N_CHUNK)]
        so_sem = [sem(f"so{b}") for b in range(B)]
        pe_sem = sem("pe_sem")
        cp_sem = sem("cp_sem")
        dve_sem = sem("dve_sem")
        dbg_sem = sem("dbg_sem") if paranoid else None

        mw_sb = sb("mw_sb", [Cp1, Cp1], f16)
        x_sb = sb("x_sb", [Cp1, NCOL * NS], f16)
        ut = [sb(f"ut{i}", [NS, FD], f16) for i in range(2)]
        vhat = sb("vhat", [NS, FD], f16)
        amv = sb("amv", [NS, FD], f16)
        # vp ring: written by DVE, counted slots DMA'd straight to DRAM
        vp_all = sb("vp_all", [NS, SPB * FD], f16)

        def vp_slot(tau):
            s = tau % SPB
            return vp_all[:, s * FD : (s + 1) * FD]
        # [128, 1024] = 2 psum banks; 5 blocks x 81 = 405 cols used per bank
        up = [
            ctx.enter_context(nc.psum_tensor(f"up{i}", [NS, 1024], f32))
            for i in range(3)
        ]

        def up_ap(i):
            # both bank pieces as one 3D AP for the batched ACT copy
            return bass.AP(up[i], 0, [[1024, NS], [512, 2], [1, HB]])

        def ut_ap(i):
            return bass.AP(ut[i], 0, [[FD, NS], [HB, 2], [1, HB]])

        with nc.Block() as block:

            @block.sync
            def _(sync):
                for k in range(N_CHUNK):
                    c0 = (CHUNK_ENDS[k - 1] if k else 0) * NS
                    c1 = CHUNK_ENDS[k] * NS
                    sync.dma_start(
                        x_sb[:, c0:c1], x_in[:, c0:c1]
                    ).then_inc(xs_sem[k], 16)
                for b in range(B):
                    sync.wait_ge(so_sem[b], 16)

            @block.gpsimd
            def _(gpsimd):
                # mw rides the gpsimd ring, parallel with x chunk 0
                gpsimd.dma_start(mw_sb[:], mw_in[:]).then_inc(mw_sem, 16)
                for j in range(B):
                    # vp ring slot for counted step W+j
                    gpsimd.wait_ge(dve_sem, W + j + 1)
                    s0 = (W + j) % SPB
                    gpsimd.dma_start(
                        sp_out[:, j * FD : (j + 1) * FD],
                        vp_all[:, s0 * FD : (s0 + 1) * FD],
                    ).then_inc(so_sem[j], 16)

            @block.tensor
            def _(tensor):
                tensor.wait_ge(mw_sem, 16)
                chunks_waited = 0
                for tau in range(STEPS):
                    need = _dma_gate_chunk(tau)
                    for k in range(chunks_waited, need):
                        tensor.wait_ge(xs_sem[k], 16)
                    chunks_waited = max(chunks_waited, need)
                    if tau >= 3:
                        tensor.wait_ge(cp_sem, tau - 2)
                    for b in range(NB):
                        t = b * B - W + tau
                        pos = 0 if t < 0 else int(T_POS[t])
                        h, off = divmod(b, 5)
                        c0 = h * 512 + off * Cp1
                        i = tensor.matmul(
                            up[tau % 3][:, c0 : c0 + Cp1],
                            x_sb[:, pos * NS : (pos + 1) * NS],
                            mw_sb[:],
                            start=True,
                            stop=True,
                        )
                        if b == NB - 1:
                            i.then_inc(pe_sem)

            @block.vector
            def _(vector):
                ndbg = 0

                def dbg(inst):
                    nonlocal ndbg
                    if paranoid:
                        inst.then_inc(dbg_sem)
                        ndbg += 1

                def dbg_wait(vector):
                    if paranoid:
                        vector.wait_ge(dbg_sem, ndbg)

                dbg(vector.memset(vhat[:], 0.0))
                for tau in range(STEPS):
                    vector.wait_ge(cp_sem, tau + 1)
                    if tau - SPB >= W:
                        # ring slot reuse: its DMA-out must be done
                        vector.wait_ge(so_sem[tau - SPB - W], 16)
                    dbg_wait(vector)
                    if paranoid:
                        vector.wait_ge(dve_sem, tau)
                    dbg(
                        vector.tensor_tensor(
                            vp_slot(tau), vhat[:], ut[tau % 2][:],
                            op=mybir.AluOpType.add,
                        )
                    )
                    dbg_wait(vector)
                    dbg(
                        vector.tensor_scalar(
                            amv[:], vp_slot(tau), float(V_TH), float(a_val),
                            op0=mybir.AluOpType.is_lt, op1=mybir.AluOpType.mult,
                        )
                    )
                    dbg_wait(vector)
                    vector.tensor_tensor(
                        vhat[:], vp_slot(tau), amv[:],
                        op=mybir.AluOpType.mult,
                    ).then_inc(dve_sem)

            @block.scalar
            def _(scalar):
                for tau in range(STEPS):
                    scalar.wait_ge(pe_sem, tau + 1)
                    if tau >= 2:
                        scalar.wait_ge(dve_sem, tau - 1)
                    scalar.copy(ut_ap(tau % 2), up_ap(tau % 3)).then_inc(cp_sem)
    return nc


def _prep_mw(conv_w, conv_b, bn_gamma, bn_beta, bn_mean, bn_var, d):
    inv = np.asarray(bn_gamma, np.float32) / np.sqrt(
        np.asarray(bn_var, np.float32) + np.float32(EPS)
    )
    w = np.asarray(conv_w, np.float32)[0, 0, :, 0]  # (64,)
    M = np.zeros((Cp1, C), np.float32)
    for h in range(Cp1):
        lo = max(0, h - 32)
        hi = min(C, h + 32)
        M[h, lo:hi] = w[lo - h + 32 : hi - h + 32]
    Mpp = (np.float32(d) * inv)[:, None] * M  # (81, 80)
    bias = np.float32(d) * (
        inv * np.float32(np.asarray(conv_b, np.float32)[0])
        + np.asarray(bn_beta, np.float32)
        - np.asarray(bn_mean, np.float32) * inv
    )
    return np.concatenate([Mpp.T, bias[None, :]], axis=0).astype(np.float16)  # (81,81)


def prep_inputs(x, conv_w, conv_b, bn_gamma, bn_beta, bn_mean, bn_var, plif_w):
    """Host-side input prep shared by kernel() and the timed rerun."""
    x = np.ascontiguousarray(np.asarray(x, np.float32))
    d = float(1.0 / (1.0 + np.exp(-np.float64(np.asarray(plif_w)))))
    a_val = 1.0 - d
    MW = _prep_mw(conv_w, conv_b, bn_gamma, bn_beta, bn_mean, bn_var, d)

    x_aug = np.concatenate([x, np.ones((N, 1, T), np.float32)], axis=1).astype(
        np.float16
    )  # (N, 81, T)
    in_maps = []
    for i in range(N_CORES):
        xs = x_aug[i * NS : (i + 1) * NS]             # (128, 81, 500)
        xs_t = xs.transpose(1, 2, 0)                  # (81, 500, 128)
        xt = np.zeros((Cp1, NCOL, NS), np.float16)
        xt[:, 1:, :] = xs_t[:, T_ORDER, :]
        in_maps.append(
            {"x": np.ascontiguousarray(xt.reshape(Cp1, NCOL * NS)), "mw": MW}
        )
    return in_maps, a_val


def finish_output(results, fc_w, fc_b):
    """Host-side: vp tiles -> spikes -> spike counts -> features -> linear."""
    vp = np.concatenate([r["sp"] for r in results], axis=0)  # (N, B*FD) f16
    s = (vp >= np.float16(V_TH)).reshape(N, B, NB, Cp1)
    feat = s.sum(axis=(1, 2), dtype=np.float32) / np.float32(T)  # (N, 81)
    out = feat @ np.asarray(fc_w, np.float32).T + np.asarray(fc_b, np.float32)
    return out.astype(np.float32)


def get_program(a_val, paranoid=False):
    key = (round(a_val, 12), paranoid)
    if key not in _PROGRAM_CACHE:
        _PROGRAM_CACHE[key] = _build_program(a_val, paranoid)
    return _PROGRAM_CACHE[key]


def kernel(x, conv_w, conv_b, bn_gamma, bn_beta, bn_mean, bn_var, plif_w, fc_w, fc_b):
    in_maps, a_val = prep_inputs(
        x, conv_w, conv_b, bn_gamma, bn_beta, bn_mean, bn_var, plif_w
    )
    nc = get_program(a_val)
    res = run_bass_kernel_spmd(nc, in_maps, list(range(N_CORES)))
    return finish_output(res.results, fc_w, fc_b)


# revision 4
# speedup vs baseline: 1.0081x; 1.0081x over previous
"""Redesigned PLIF/conv kernel for TRN2.

Per scan step tau (55 steps = 5 warmup + 50 counted), all engines pipelined:
  - PE: 10 matmuls (one per scan block) compute u = MW^T @ x_col into PSUM
        (f16 inputs, fp32 accumulate), 5+5 blocks across two banks.
  - ACT: one batched PSUM->SBUF copy converts u to f16 (ut double buffer).
  - DVE (f16, 2x/4x perf modes): vp = vhat + ut; amv = (vp<1)*a;
        vhat = vp*amv  -- PLIF charge/fire/reset, state carried in vhat.
  - DMA out (gpsimd ring): each counted vp tile streams to DRAM;
        host thresholds vp >= 1 and sums spike counts (exact same boundary
        as the kernel's reset decision).

x is streamed in "first-need" permuted column order so the scan starts
within ~2us of kernel start; one zero column at stream position 0 feeds
block 0's warmup.
"""
import sys

sys.path.insert(0, "/opt/trn_rl_repo")

import contextlib

import numpy as np

import concourse.bass as bass
import concourse.mybir as mybir
from concourse.bass_utils import run_bass_kernel_spmd

# ---- problem constants ----------------------------------------------------
N_CORES = 8
N, C, T = 1024, 80, 500
Cp1 = C + 1                  # 81
NS = N // N_CORES            # 128 samples per core
NB, B, W = 10, 50, 5         # scan blocks, counted steps per block, warmup
STEPS = W + B                # 55
FD = NB * Cp1                # 810 free-dim elements per scan tile
HB = FD // 2                 # 405 per psum bank piece
NCOL = T + 1                 # stream columns incl. leading zero column
EPS = 1e-5
V_TH = 1.0

# x stream DMA chunk ends (in columns): small first chunk so step 0's
# matmuls (needing 1 + 9 columns) start as soon as possible
_ends = [10]
while _ends[-1] < NCOL:
    _ends.append(min(NCOL, _ends[-1] + 25))
CHUNK_ENDS = _ends
N_CHUNK = len(CHUNK_ENDS)    # 21
SPB = 10                     # vp-tile ring depth; one output DMA per tile

_PROGRAM_CACHE = {}


def _first_need_order():
    """Column stream order: position 0 is the zero column; real columns
    sorted by the first scan step that consumes them."""
    t = np.arange(T)
    c = t % B
    b = t // B
    tau_first = np.where((c >= B - W) & (b + 1 <= NB - 1), c - (B - W), W + c)
    order = np.argsort(tau_first, kind="stable")  # t values in stream order
    pos = np.empty(T, np.int64)
    pos[order] = np.arange(T) + 1  # +1 for zero column at position 0
    return order, pos


T_ORDER, T_POS = _first_need_order()


def _dma_gate_chunk(tau):
    """Number of x DMA chunks needed before step tau's matmuls."""
    if tau < W:
        p = 1 + (NB - 1) * (tau + 1)
    elif tau < B:
        p = 1 + (NB - 1) * W + NB * (tau - W + 1)
    else:
        p = 1 + (NB - 1) * W + NB * (B - W) + (tau - B + 1)
    p = min(p, NCOL)
    for k, e in enumerate(CHUNK_ENDS):
        if e >= p:
            return k + 1
    return N_CHUNK


def _build_program(a_val: float, paranoid: bool = False):
    """paranoid=True threads every same-engine DVE dependency through a real
    semaphore so CoreSim's race detector (which doesn't credit same-engine
    program order) can verify the cross-engine sync. Data-identical."""
    f32 = mybir.dt.float32
    f16 = mybir.dt.float16
    nc = bass.Bass()
    x_in = nc.dram_tensor("x", [Cp1, NCOL * NS], f16, kind="ExternalInput")
    mw_in = nc.dram_tensor("mw", [Cp1, Cp1], f16, kind="ExternalInput")
    sp_out = nc.dram_tensor("sp", [NS, B * FD], f16, kind="ExternalOutput")

    with contextlib.ExitStack() as ctx:
        def sem(name):
            return ctx.enter_context(nc.semaphore(name))

        def sb(name, shape, dtype):
            return ctx.enter_context(nc.sbuf_tensor(name, shape, dtype))

        mw_sem = sem("mw_sem")
        # one sem per DMA so the race detector sees unambiguous counts
        xs_sem = [sem(f"xs{k}") for k in range(N_CHUNK)]
        so_sem = [sem(f"so{b}") for b in range(B)]
        pe_sem = sem("pe_sem")
        cp_sem = sem("cp_sem")
        dve_sem = sem("dve_sem")
        dbg_sem = sem("dbg_sem") if paranoid else None

        mw_sb = sb("mw_sb", [Cp1, Cp1], f16)
        x_sb = sb("x_sb", [Cp1, NCOL * NS], f16)
        ut = [sb(f"ut{i}", [NS, FD], f16) for i in range(3)]
        vhat = sb("vhat", [NS, FD], f16)
        amv = sb("amv", [NS, FD], f16)
        # vp ring: written by DVE, counted slots DMA'd straight to DRAM
        vp_all = sb("vp_all", [NS, SPB * FD], f16)

        def vp_slot(tau):
            s = tau % SPB
            return vp_all[:, s * FD : (s + 1) * FD]
        # [128, 1024] = 2 psum banks; 5 blocks x 81 = 405 cols used per bank;
        # 4 step-buffers use all 8 banks for max PE run-ahead
        up = [
            ctx.enter_context(nc.psum_tensor(f"up{i}", [NS, 1024], f32))
            for i in range(4)
        ]

        def up_ap(i):
            # both bank pieces as one 3D AP for the batched ACT copy
            return bass.AP(up[i], 0, [[1024, NS], [512, 2], [1, HB]])

        def ut_ap(i):
            return bass.AP(ut[i], 0, [[FD, NS], [HB, 2], [1, HB]])

        with nc.Block() as block:

            @block.sync
            def _(sync):
                for k in range(N_CHUNK):
                    c0 = (CHUNK_ENDS[k - 1] if k else 0) * NS
                    c1 = CHUNK_ENDS[k] * NS
                    sync.dma_start(
                        x_sb[:, c0:c1], x_in[:, c0:c1]
                    ).then_inc(xs_sem[k], 16)
                for b in range(B):
                    sync.wait_ge(so_sem[b], 16)

            @block.gpsimd
            def _(gpsimd):
                # mw rides the gpsimd ring, parallel with x chunk 0
                gpsimd.dma_start(mw_sb[:], mw_in[:]).then_inc(mw_sem, 16)
                for j in range(B):
                    # vp ring slot for counted step W+j
                    gpsimd.wait_ge(dve_sem, W + j + 1)
                    s0 = (W + j) % SPB
                    gpsimd.dma_start(
                        sp_out[:, j * FD : (j + 1) * FD],
                        vp_all[:, s0 * FD : (s0 + 1) * FD],
                    ).then_inc(so_sem[j], 16)

            @block.tensor
            def _(tensor):
                tensor.wait_ge(mw_sem, 16)
                chunks_waited = 0
                for tau in range(STEPS):
                    need = _dma_gate_chunk(tau)
                    for k in range(chunks_waited, need):
                        tensor.wait_ge(xs_sem[k], 16)
                    chunks_waited = max(chunks_waited, need)
                    if tau >= 4:
                        tensor.wait_ge(cp_sem, tau - 3)
                    for b in range(NB):
                        t = b * B - W + tau
                        pos = 0 if t < 0 else int(T_POS[t])
                        h, off = divmod(b, 5)
                        c0 = h * 512 + off * Cp1
                        i = tensor.matmul(
                            up[tau % 4][:, c0 : c0 + Cp1],
                            x_sb[:, pos * NS : (pos + 1) * NS],
                            mw_sb[:],
                            start=True,
                            stop=True,
                        )
                        if b == NB - 1:
                            i.then_inc(pe_sem)

            @block.vector
            def _(vector):
                ndbg = 0

                def dbg(inst):
                    nonlocal ndbg
                    if paranoid:
                        inst.then_inc(dbg_sem)
                        ndbg += 1

                def dbg_wait(vector):
                    if paranoid:
                        vector.wait_ge(dbg_sem, ndbg)

                for tau in range(STEPS):
                    vector.wait_ge(cp_sem, tau + 1)
                    if tau - SPB >= W:
                        # ring slot reuse: its DMA-out must be done
                        vector.wait_ge(so_sem[tau - SPB - W], 16)
                    dbg_wait(vector)
                    if paranoid:
                        vector.wait_ge(dve_sem, tau)
                    if tau == 0:
                        # vhat starts at 0, so vp == ut: skip the add (and
                        # the vhat memset -- first write is this step's mult)
                        vp_t = ut[0][:]
                    else:
                        vp_t = vp_slot(tau)
                        i = vector.tensor_tensor(
                            vp_t, vhat[:], ut[tau % 3][:],
                            op=mybir.AluOpType.add,
                        )
                        if tau == STEPS - 1:
                            # last step: vhat/amv are never consumed
                            i.then_inc(dve_sem)
                            break
                        dbg(i)
                    dbg_wait(vector)
                    dbg(
                        vector.tensor_scalar(
                            amv[:], vp_t, float(V_TH), float(a_val),
                            op0=mybir.AluOpType.is_lt, op1=mybir.AluOpType.mult,
                        )
                    )
                    dbg_wait(vector)
                    vector.tensor_tensor(
                        vhat[:], vp_t, amv[:],
                        op=mybir.AluOpType.mult,
                    ).then_inc(dve_sem)

            @block.scalar
            def _(scalar):
                for tau in range(STEPS):
                    scalar.wait_ge(pe_sem, tau + 1)
                    if tau >= 3:
                        scalar.wait_ge(dve_sem, tau - 2)
                    scalar.copy(ut_ap(tau % 3), up_ap(tau % 4)).then_inc(cp_sem)
    return nc


def _prep_mw(conv_w, conv_b, bn_gamma, bn_beta, bn_mean, bn_var, d):
    inv = np.asarray(bn_gamma, np.float32) / np.sqrt(
        np.asarray(bn_var, np.float32) + np.float32(EPS)
    )
    w = np.asarray(conv_w, np.float32)[0, 0, :, 0]  # (64,)
    M = np.zeros((Cp1, C), np.float32)
    for h in range(Cp1):
        lo = max(0, h - 32)
        hi = min(C, h + 32)
        M[h, lo:hi] = w[lo - h + 32 : hi - h + 32]
    Mpp = (np.float32(d) * inv)[:, None] * M  # (81, 80)
    bias = np.float32(d) * (
        inv * np.float32(np.asarray(conv_b, np.float32)[0])
        + np.asarray(bn_beta, np.float32)
        - np.asarray(bn_mean, np.float32) * inv
    )
    return np.concatenate([Mpp.T, bias[None, :]], axis=0).astype(np.float16)  # (81,81)


def prep_inputs(x, conv_w, conv_b, bn_gamma, bn_beta, bn_mean, bn_var, plif_w):
    """Host-side input prep shared by kernel() and the timed rerun."""
    x = np.ascontiguousarray(np.asarray(x, np.float32))
    d = float(1.0 / (1.0 + np.exp(-np.float64(np.asarray(plif_w)))))
    a_val = 1.0 - d
    MW = _prep_mw(conv_w, conv_b, bn_gamma, bn_beta, bn_mean, bn_var, d)

    x_aug = np.concatenate([x, np.ones((N, 1, T), np.float32)], axis=1).astype(
        np.float16
    )  # (N, 81, T)
    in_maps = []
    for i in range(N_CORES):
        xs = x_aug[i * NS : (i + 1) * NS]             # (128, 81, 500)
        xs_t = xs.transpose(1, 2, 0)                  # (81, 500, 128)
        xt = np.zeros((Cp1, NCOL, NS), np.float16)
        xt[:, 1:, :] = xs_t[:, T_ORDER, :]
        in_maps.append(
            {"x": np.ascontiguousarray(xt.reshape(Cp1, NCOL * NS)), "mw": MW}
        )
    return in_maps, a_val


def finish_output(results, fc_w, fc_b):
    """Host-side: vp tiles -> spikes -> spike counts -> features -> linear."""
    vp = np.concatenate([r["sp"] for r in results], axis=0)  # (N, B*FD) f16
    s = (vp >= np.float16(V_TH)).reshape(N, B, NB, Cp1)
    feat = s.sum(axis=(1, 2), dtype=np.float32) / np.float32(T)  # (N, 81)
    out = feat @ np.asarray(fc_w, np.float32).T + np.asarray(fc_b, np.float32)
    return out.astype(np.float32)


def get_program(a_val, paranoid=False):
    key = (round(a_val, 12), paranoid)
    if key not in _PROGRAM_CACHE:
        _PROGRAM_CACHE[key] = _build_program(a_val, paranoid)
    return _PROGRAM_CACHE[key]


def kernel(x, conv_w, conv_b, bn_gamma, bn_beta, bn_mean, bn_var, plif_w, fc_w, fc_b):
    in_maps, a_val = prep_inputs(
        x, conv_w, conv_b, bn_gamma, bn_beta, bn_mean, bn_var, plif_w
    )
    nc = get_program(a_val)
    res = run_bass_kernel_spmd(nc, in_maps, list(range(N_CORES)))
    return finish_output(res.results, fc_w, fc_b)


# revision 5
# speedup vs baseline: 1.0225x; 1.0143x over previous
"""Redesigned PLIF/conv kernel for TRN2.

Per scan step tau (55 steps = 5 warmup + 50 counted), all engines pipelined:
  - PE: 10 matmuls (one per scan block) compute u = MW^T @ x_col into PSUM
        (f16 inputs, fp32 accumulate), 5+5 blocks across two banks.
  - ACT: one batched PSUM->SBUF copy converts u to f16 (ut double buffer).
  - DVE (f16, 2x/4x perf modes): vp = vhat + ut; amv = (vp<1)*a;
        vhat = vp*amv  -- PLIF charge/fire/reset, state carried in vhat.
  - DMA out (gpsimd ring): each counted vp tile streams to DRAM;
        host thresholds vp >= 1 and sums spike counts (exact same boundary
        as the kernel's reset decision).

x is streamed in "first-need" permuted column order so the scan starts
within ~2us of kernel start; one zero column at stream position 0 feeds
block 0's warmup.
"""
import sys

sys.path.insert(0, "/opt/trn_rl_repo")

import contextlib

import numpy as np

import concourse.bass as bass
import concourse.mybir as mybir
from concourse.bass_utils import run_bass_kernel_spmd

# ---- problem constants ----------------------------------------------------
N_CORES = 8
N, C, T = 1024, 80, 500
Cp1 = C + 1                  # 81
NS = N // N_CORES            # 128 samples per core
NB, B, W = 10, 50, 4         # scan blocks, counted steps per block, warmup
STEPS = W + B                # 54
FD = NB * Cp1                # 810 free-dim elements per scan tile
HB = FD // 2                 # 405 per psum bank piece
NCOL = T + 1                 # stream columns incl. leading zero column
EPS = 1e-5
V_TH = 1.0

# x stream DMA chunk ends (in columns): small first chunk so step 0's
# matmuls (needing 1 + 9 columns) start as soon as possible
_ends = [10]
while _ends[-1] < NCOL:
    _ends.append(min(NCOL, _ends[-1] + 25))
CHUNK_ENDS = _ends
N_CHUNK = len(CHUNK_ENDS)    # 21
SPB = 10                     # vp-tile ring depth; one output DMA per tile

_PROGRAM_CACHE = {}


def _first_need_order():
    """Column stream order: position 0 is the zero column; real columns
    sorted by the first scan step that consumes them."""
    t = np.arange(T)
    c = t % B
    b = t // B
    tau_first = np.where((c >= B - W) & (b + 1 <= NB - 1), c - (B - W), W + c)
    order = np.argsort(tau_first, kind="stable")  # t values in stream order
    pos = np.empty(T, np.int64)
    pos[order] = np.arange(T) + 1  # +1 for zero column at position 0
    return order, pos


T_ORDER, T_POS = _first_need_order()


def _dma_gate_chunk(tau):
    """Number of x DMA chunks needed before step tau's matmuls."""
    if tau < W:
        p = 1 + (NB - 1) * (tau + 1)
    elif tau < B:
        p = 1 + (NB - 1) * W + NB * (tau - W + 1)
    else:
        p = 1 + (NB - 1) * W + NB * (B - W) + (tau - B + 1)
    p = min(p, NCOL)
    for k, e in enumerate(CHUNK_ENDS):
        if e >= p:
            return k + 1
    return N_CHUNK


def _build_program(a_val: float, paranoid: bool = False):
    """paranoid=True threads every same-engine DVE dependency through a real
    semaphore so CoreSim's race detector (which doesn't credit same-engine
    program order) can verify the cross-engine sync. Data-identical."""
    f32 = mybir.dt.float32
    f16 = mybir.dt.float16
    nc = bass.Bass()
    x_in = nc.dram_tensor("x", [Cp1, NCOL * NS], f16, kind="ExternalInput")
    mw_in = nc.dram_tensor("mw", [Cp1, Cp1], f16, kind="ExternalInput")
    sp_out = nc.dram_tensor("sp", [NS, B * FD], f16, kind="ExternalOutput")

    with contextlib.ExitStack() as ctx:
        def sem(name):
            return ctx.enter_context(nc.semaphore(name))

        def sb(name, shape, dtype):
            return ctx.enter_context(nc.sbuf_tensor(name, shape, dtype))

        mw_sem = sem("mw_sem")
        # one sem per DMA so the race detector sees unambiguous counts
        xs_sem = [sem(f"xs{k}") for k in range(N_CHUNK)]
        so_sem = [sem(f"so{b}") for b in range(B)]
        pe_sem = sem("pe_sem")
        warm_sem = sem("warm_sem")
        cp_sem = sem("cp_sem")
        dve_sem = sem("dve_sem")
        dbg_sem = sem("dbg_sem") if paranoid else None

        mw_sb = sb("mw_sb", [Cp1, Cp1], f16)
        x_sb = sb("x_sb", [Cp1, NCOL * NS], f16)
        ut = [sb(f"ut{i}", [NS, FD], f16) for i in range(3)]
        vhat = sb("vhat", [NS, FD], f16)
        amv = sb("amv", [NS, FD], f16)
        # vp ring: written by DVE, counted slots DMA'd straight to DRAM
        vp_all = sb("vp_all", [NS, SPB * FD], f16)

        def vp_slot(tau):
            s = tau % SPB
            return vp_all[:, s * FD : (s + 1) * FD]
        # [128, 1024] = 2 psum banks; 5 blocks x 81 = 405 cols used per bank;
        # 4 step-buffers use all 8 banks for max PE run-ahead
        up = [
            ctx.enter_context(nc.psum_tensor(f"up{i}", [NS, 1024], f32))
            for i in range(4)
        ]

        def up_ap(i):
            # both bank pieces as one 3D AP for the batched ACT copy
            return bass.AP(up[i], 0, [[1024, NS], [512, 2], [1, HB]])

        def ut_ap(i):
            return bass.AP(ut[i], 0, [[FD, NS], [HB, 2], [1, HB]])

        with nc.Block() as block:

            @block.sync
            def _(sync):
                for k in range(N_CHUNK):
                    c0 = (CHUNK_ENDS[k - 1] if k else 0) * NS
                    c1 = CHUNK_ENDS[k] * NS
                    sync.dma_start(
                        x_sb[:, c0:c1], x_in[:, c0:c1]
                    ).then_inc(xs_sem[k], 16)
                for b in range(B):
                    sync.wait_ge(so_sem[b], 16)

            @block.gpsimd
            def _(gpsimd):
                # mw rides the gpsimd ring, parallel with x chunk 0
                gpsimd.dma_start(mw_sb[:], mw_in[:]).then_inc(mw_sem, 16)
                for j in range(B):
                    # vp ring slot for counted step W+j
                    gpsimd.wait_ge(dve_sem, W + j + 1)
                    s0 = (W + j) % SPB
                    gpsimd.dma_start(
                        sp_out[:, j * FD : (j + 1) * FD],
                        vp_all[:, s0 * FD : (s0 + 1) * FD],
                    ).then_inc(so_sem[j], 16)

            @block.tensor
            def _(tensor):
                tensor.wait_ge(mw_sem, 16)
                # dummy matmuls while the first x chunk is in flight: keeps
                # the PE pipeline warm (p-state ramp) for the real stream.
                # up[3] garbage is overwritten with start=True at tau=3.
                for d in range(16):
                    i = tensor.matmul(
                        up[3][0:Cp1, 0:Cp1], mw_sb[:], mw_sb[:],
                        start=True, stop=True,
                    )
                    if d == 15:
                        i.then_inc(warm_sem)
                chunks_waited = 0
                for tau in range(STEPS):
                    need = _dma_gate_chunk(tau)
                    for k in range(chunks_waited, need):
                        tensor.wait_ge(xs_sem[k], 16)
                    chunks_waited = max(chunks_waited, need)
                    if tau == 3:
                        tensor.wait_ge(warm_sem, 1)
                    if tau >= 4:
                        tensor.wait_ge(cp_sem, tau - 3)
                    for b in range(NB):
                        t = b * B - W + tau
                        pos = 0 if t < 0 else int(T_POS[t])
                        h, off = divmod(b, 5)
                        c0 = h * 512 + off * Cp1
                        i = tensor.matmul(
                            up[tau % 4][:, c0 : c0 + Cp1],
                            x_sb[:, pos * NS : (pos + 1) * NS],
                            mw_sb[:],
                            start=True,
                            stop=True,
                        )
                        if b == NB - 1:
                            i.then_inc(pe_sem)

            @block.vector
            def _(vector):
                ndbg = 0

                def dbg(inst):
                    nonlocal ndbg
                    if paranoid:
                        inst.then_inc(dbg_sem)
                        ndbg += 1

                def dbg_wait(vector):
                    if paranoid:
                        vector.wait_ge(dbg_sem, ndbg)

                for tau in range(STEPS):
                    vector.wait_ge(cp_sem, tau + 1)
                    if tau - SPB >= W:
                        # ring slot reuse: its DMA-out must be done
                        vector.wait_ge(so_sem[tau - SPB - W], 16)
                    dbg_wait(vector)
                    if paranoid:
                        vector.wait_ge(dve_sem, tau)
                    if tau == 0:
                        # vhat starts at 0, so vp == ut: skip the add (and
                        # the vhat memset -- first write is this step's mult)
                        vp_t = ut[0][:]
                    else:
                        vp_t = vp_slot(tau)
                        i = vector.tensor_tensor(
                            vp_t, vhat[:], ut[tau % 3][:],
                            op=mybir.AluOpType.add,
                        )
                        if tau == STEPS - 1:
                            # last step: vhat/amv are never consumed
                            i.then_inc(dve_sem)
                            break
                        dbg(i)
                    dbg_wait(vector)
                    dbg(
                        vector.tensor_scalar(
                            amv[:], vp_t, float(V_TH), float(a_val),
                            op0=mybir.AluOpType.is_lt, op1=mybir.AluOpType.mult,
                        )
                    )
                    dbg_wait(vector)
                    vector.tensor_tensor(
                        vhat[:], vp_t, amv[:],
                        op=mybir.AluOpType.mult,
                    ).then_inc(dve_sem)

            @block.scalar
            def _(scalar):
                for tau in range(STEPS):
                    scalar.wait_ge(pe_sem, tau + 1)
                    if tau >= 3:
                        scalar.wait_ge(dve_sem, tau - 2)
                    scalar.copy(ut_ap(tau % 3), up_ap(tau % 4)).then_inc(cp_sem)
    return nc


def _prep_mw(conv_w, conv_b, bn_gamma, bn_beta, bn_mean, bn_var, d):
    inv = np.asarray(bn_gamma, np.float32) / np.sqrt(
        np.asarray(bn_var, np.float32) + np.float32(EPS)
    )
    w = np.asarray(conv_w, np.float32)[0, 0, :, 0]  # (64,)
    M = np.zeros((Cp1, C), np.float32)
    for h in range(Cp1):
        lo = max(0, h - 32)
        hi = min(C, h + 32)
        M[h, lo:hi] = w[lo - h + 32 : hi - h + 32]
    Mpp = (np.float32(d) * inv)[:, None] * M  # (81, 80)
    bias = np.float32(d) * (
        inv * np.float32(np.asarray(conv_b, np.float32)[0])
        + np.asarray(bn_beta, np.float32)
        - np.asarray(bn_mean, np.float32) * inv
    )
    return np.concatenate([Mpp.T, bias[None, :]], axis=0).astype(np.float16)  # (81,81)


def prep_inputs(x, conv_w, conv_b, bn_gamma, bn_beta, bn_mean, bn_var, plif_w):
    """Host-side input prep shared by kernel() and the timed rerun."""
    x = np.ascontiguousarray(np.asarray(x, np.float32))
    d = float(1.0 / (1.0 + np.exp(-np.float64(np.asarray(plif_w)))))
    a_val = 1.0 - d
    MW = _prep_mw(conv_w, conv_b, bn_gamma, bn_beta, bn_mean, bn_var, d)

    x_aug = np.concatenate([x, np.ones((N, 1, T), np.float32)], axis=1).astype(
        np.float16
    )  # (N, 81, T)
    in_maps = []
    for i in range(N_CORES):
        xs = x_aug[i * NS : (i + 1) * NS]             # (128, 81, 500)
        xs_t = xs.transpose(1, 2, 0)                  # (81, 500, 128)
        xt = np.zeros((Cp1, NCOL, NS), np.float16)
        xt[:, 1:, :] = xs_t[:, T_ORDER, :]
        in_maps.append(
            {"x": np.ascontiguousarray(xt.reshape(Cp1, NCOL * NS)), "mw": MW}
        )
    return in_maps, a_val


def finish_output(results, fc_w, fc_b):
    """Host-side: vp tiles -> spikes -> spike counts -> features -> linear."""
    vp = np.concatenate([r["sp"] for r in results], axis=0)  # (N, B*FD) f16
    s = (vp >= np.float16(V_TH)).reshape(N, B, NB, Cp1)
    feat = s.sum(axis=(1, 2), dtype=np.float32) / np.float32(T)  # (N, 81)
    out = feat @ np.asarray(fc_w, np.float32).T + np.asarray(fc_b, np.float32)
    return out.astype(np.float32)


def get_program(a_val, paranoid=False):
    key = (round(a_val, 12), paranoid)
    if key not in _PROGRAM_CACHE:
        _PROGRAM_CACHE[key] = _build_program(a_val, paranoid)
    return _PROGRAM_CACHE[key]


def kernel(x, conv_w, conv_b, bn_gamma, bn_beta, bn_mean, bn_var, plif_w, fc_w, fc_b):
    in_maps, a_val = prep_inputs(
        x, conv_w, conv_b, bn_gamma, bn_beta, bn_mean, bn_var, plif_w
    )
    nc = get_program(a_val)
    res = run_bass_kernel_spmd(nc, in_maps, list(range(N_CORES)))
    return finish_output(res.results, fc_w, fc_b)


# revision 7
# speedup vs baseline: 1.0358x; 1.0131x over previous
"""Redesigned PLIF/conv kernel for TRN2.

Per scan step tau (55 steps = 5 warmup + 50 counted), all engines pipelined:
  - PE: 10 matmuls (one per scan block) compute u = MW^T @ x_col into PSUM
        (f16 inputs, fp32 accumulate), 5+5 blocks across two banks.
  - ACT: one batched PSUM->SBUF copy converts u to f16 (ut double buffer).
  - DVE (f16, 2x/4x perf modes): vp = vhat + ut; amv = (vp<1)*a;
        vhat = vp*amv  -- PLIF charge/fire/reset, state carried in vhat.
  - DMA out (gpsimd ring): each counted vp tile streams to DRAM;
        host thresholds vp >= 1 and sums spike counts (exact same boundary
        as the kernel's reset decision).

x is streamed in "first-need" permuted column order so the scan starts
within ~2us of kernel start; one zero column at stream position 0 feeds
block 0's warmup.
"""
import sys

sys.path.insert(0, "/opt/trn_rl_repo")

import contextlib

import numpy as np

import concourse.bass as bass
import concourse.mybir as mybir
from concourse.bass_utils import run_bass_kernel_spmd

# ---- problem constants ----------------------------------------------------
N_CORES = 8
N, C, T = 1024, 80, 500
Cp1 = C + 1                  # 81
NS = N // N_CORES            # 128 samples per core
NB, B, W = 10, 50, 3         # scan blocks, counted steps per block, warmup
STEPS = W + B                # 53
FD = NB * Cp1                # 810 free-dim elements per scan tile
HB = FD // 2                 # 405 per psum bank piece
NCOL = T + 1                 # stream columns incl. leading zero column
EPS = 1e-5
V_TH = 1.0

# x stream DMA chunk ends (in columns): the first chunks match each early
# scan step's exact column need (9-10 new columns/step) so the pipeline
# ramps without waiting on coarse chunks; steady state uses 25-col chunks
_ends = [1 + (NB - 1) * (t + 1) for t in range(W)]          # warmup steps
_ends += [_ends[-1] + NB * (j + 1) for j in range(3)]       # first counted
while _ends[-1] < NCOL:
    _ends.append(min(NCOL, _ends[-1] + 25))
CHUNK_ENDS = _ends
N_CHUNK = len(CHUNK_ENDS)
SPB = 10                     # vp-tile ring depth; one output DMA per tile

_PROGRAM_CACHE = {}


def _first_need_order():
    """Column stream order: position 0 is the zero column; real columns
    sorted by the first scan step that consumes them."""
    t = np.arange(T)
    c = t % B
    b = t // B
    tau_first = np.where((c >= B - W) & (b + 1 <= NB - 1), c - (B - W), W + c)
    order = np.argsort(tau_first, kind="stable")  # t values in stream order
    pos = np.empty(T, np.int64)
    pos[order] = np.arange(T) + 1  # +1 for zero column at position 0
    return order, pos


T_ORDER, T_POS = _first_need_order()


def _dma_gate_chunk(tau):
    """Number of x DMA chunks needed before step tau's matmuls."""
    if tau < W:
        p = 1 + (NB - 1) * (tau + 1)
    elif tau < B:
        p = 1 + (NB - 1) * W + NB * (tau - W + 1)
    else:
        p = 1 + (NB - 1) * W + NB * (B - W) + (tau - B + 1)
    p = min(p, NCOL)
    for k, e in enumerate(CHUNK_ENDS):
        if e >= p:
            return k + 1
    return N_CHUNK


def _build_program(a_val: float, paranoid: bool = False):
    """paranoid=True threads every same-engine DVE dependency through a real
    semaphore so CoreSim's race detector (which doesn't credit same-engine
    program order) can verify the cross-engine sync. Data-identical."""
    f32 = mybir.dt.float32
    f16 = mybir.dt.float16
    nc = bass.Bass()
    x_in = nc.dram_tensor("x", [Cp1, NCOL * NS], f16, kind="ExternalInput")
    mw_in = nc.dram_tensor("mw", [Cp1, Cp1], f16, kind="ExternalInput")
    sp_out = nc.dram_tensor("sp", [NS, B * FD], f16, kind="ExternalOutput")

    with contextlib.ExitStack() as ctx:
        def sem(name):
            return ctx.enter_context(nc.semaphore(name))

        def sb(name, shape, dtype):
            return ctx.enter_context(nc.sbuf_tensor(name, shape, dtype))

        mw_sem = sem("mw_sem")
        # one sem per DMA so the race detector sees unambiguous counts
        xs_sem = [sem(f"xs{k}") for k in range(N_CHUNK)]
        so_sem = [sem(f"so{b}") for b in range(B)]
        pe_sem = sem("pe_sem")
        warm_sem = sem("warm_sem")
        cp_sem = sem("cp_sem")
        dve_sem = sem("dve_sem")
        dbg_sem = sem("dbg_sem") if paranoid else None

        mw_sb = sb("mw_sb", [Cp1, Cp1], f16)
        x_sb = sb("x_sb", [Cp1, NCOL * NS], f16)
        ut = [sb(f"ut{i}", [NS, FD], f16) for i in range(3)]
        vhat = sb("vhat", [NS, FD], f16)
        amv = sb("amv", [NS, FD], f16)
        # vp ring: written by DVE, counted slots DMA'd straight to DRAM
        vp_all = sb("vp_all", [NS, SPB * FD], f16)

        def vp_slot(tau):
            s = tau % SPB
            return vp_all[:, s * FD : (s + 1) * FD]
        # [128, 1024] = 2 psum banks; 5 blocks x 81 = 405 cols used per bank;
        # 4 step-buffers use all 8 banks for max PE run-ahead
        up = [
            ctx.enter_context(nc.psum_tensor(f"up{i}", [NS, 1024], f32))
            for i in range(4)
        ]

        def up_ap(i):
            # both bank pieces as one 3D AP for the batched ACT copy
            return bass.AP(up[i], 0, [[1024, NS], [512, 2], [1, HB]])

        def ut_ap(i):
            return bass.AP(ut[i], 0, [[FD, NS], [HB, 2], [1, HB]])

        with nc.Block() as block:

            @block.sync
            def _(sync):
                for k in range(N_CHUNK):
                    c0 = (CHUNK_ENDS[k - 1] if k else 0) * NS
                    c1 = CHUNK_ENDS[k] * NS
                    sync.dma_start(
                        x_sb[:, c0:c1], x_in[:, c0:c1]
                    ).then_inc(xs_sem[k], 16)
                for b in range(B):
                    sync.wait_ge(so_sem[b], 16)

            @block.gpsimd
            def _(gpsimd):
                # mw rides the gpsimd ring, parallel with x chunk 0
                gpsimd.dma_start(mw_sb[:], mw_in[:]).then_inc(mw_sem, 16)
                for j in range(B):
                    # vp ring slot for counted step W+j
                    gpsimd.wait_ge(dve_sem, W + j + 1)
                    s0 = (W + j) % SPB
                    gpsimd.dma_start(
                        sp_out[:, j * FD : (j + 1) * FD],
                        vp_all[:, s0 * FD : (s0 + 1) * FD],
                    ).then_inc(so_sem[j], 16)

            @block.tensor
            def _(tensor):
                tensor.wait_ge(mw_sem, 16)
                # dummy matmuls while the first x chunk is in flight: keeps
                # the PE pipeline warm (p-state ramp) for the real stream.
                # up[3] garbage is overwritten with start=True at tau=3.
                for d in range(24):
                    i = tensor.matmul(
                        up[3][0:Cp1, 0:Cp1], mw_sb[:], mw_sb[:],
                        start=True, stop=True,
                    )
                    if d == 23:
                        i.then_inc(warm_sem)
                chunks_waited = 0
                for tau in range(STEPS):
                    need = _dma_gate_chunk(tau)
                    for k in range(chunks_waited, need):
                        tensor.wait_ge(xs_sem[k], 16)
                    chunks_waited = max(chunks_waited, need)
                    if tau == 3:
                        tensor.wait_ge(warm_sem, 1)
                    if tau >= 4:
                        tensor.wait_ge(cp_sem, tau - 3)
                    for b in range(NB):
                        t = b * B - W + tau
                        pos = 0 if t < 0 else int(T_POS[t])
                        h, off = divmod(b, 5)
                        c0 = h * 512 + off * Cp1
                        i = tensor.matmul(
                            up[tau % 4][:, c0 : c0 + Cp1],
                            x_sb[:, pos * NS : (pos + 1) * NS],
                            mw_sb[:],
                            start=True,
                            stop=True,
                        )
                        if b == NB - 1:
                            i.then_inc(pe_sem)

            @block.vector
            def _(vector):
                ndbg = 0

                def dbg(inst):
                    nonlocal ndbg
                    if paranoid:
                        inst.then_inc(dbg_sem)
                        ndbg += 1

                def dbg_wait(vector):
                    if paranoid:
                        vector.wait_ge(dbg_sem, ndbg)

                for tau in range(STEPS):
                    vector.wait_ge(cp_sem, tau + 1)
                    if tau - SPB >= W:
                        # ring slot reuse: its DMA-out must be done
                        vector.wait_ge(so_sem[tau - SPB - W], 16)
                    dbg_wait(vector)
                    if paranoid:
                        vector.wait_ge(dve_sem, tau)
                    if tau == 0:
                        # vhat starts at 0, so vp == ut: skip the add (and
                        # the vhat memset -- first write is this step's mult)
                        vp_t = ut[0][:]
                    else:
                        vp_t = vp_slot(tau)
                        i = vector.tensor_tensor(
                            vp_t, vhat[:], ut[tau % 3][:],
                            op=mybir.AluOpType.add,
                        )
                        if tau == STEPS - 1:
                            # last step: vhat/amv are never consumed
                            i.then_inc(dve_sem)
                            break
                        dbg(i)
                    dbg_wait(vector)
                    dbg(
                        vector.tensor_scalar(
                            amv[:], vp_t, float(V_TH), float(a_val),
                            op0=mybir.AluOpType.is_lt, op1=mybir.AluOpType.mult,
                        )
                    )
                    dbg_wait(vector)
                    vector.tensor_tensor(
                        vhat[:], vp_t, amv[:],
                        op=mybir.AluOpType.mult,
                    ).then_inc(dve_sem)

            @block.scalar
            def _(scalar):
                for tau in range(STEPS):
                    scalar.wait_ge(pe_sem, tau + 1)
                    if tau >= 3:
                        scalar.wait_ge(dve_sem, tau - 2)
                    scalar.copy(ut_ap(tau % 3), up_ap(tau % 4)).then_inc(cp_sem)
    return nc


def _prep_mw(conv_w, conv_b, bn_gamma, bn_beta, bn_mean, bn_var, d):
    inv = np.asarray(bn_gamma, np.float32) / np.sqrt(
        np.asarray(bn_var, np.float32) + np.float32(EPS)
    )
    w = np.asarray(conv_w, np.float32)[0, 0, :, 0]  # (64,)
    M = np.zeros((Cp1, C), np.float32)
    for h in range(Cp1):
        lo = max(0, h - 32)
        hi = min(C, h + 32)
        M[h, lo:hi] = w[lo - h + 32 : hi - h + 32]
    Mpp = (np.float32(d) * inv)[:, None] * M  # (81, 80)
    bias = np.float32(d) * (
        inv * np.float32(np.asarray(conv_b, np.float32)[0])
        + np.asarray(bn_beta, np.float32)
        - np.asarray(bn_mean, np.float32) * inv
    )
    return np.concatenate([Mpp.T, bias[None, :]], axis=0).astype(np.float16)  # (81,81)


def prep_inputs(x, conv_w, conv_b, bn_gamma, bn_beta, bn_mean, bn_var, plif_w):
    """Host-side input prep shared by kernel() and the timed rerun."""
    x = np.ascontiguousarray(np.asarray(x, np.float32))
    d = float(1.0 / (1.0 + np.exp(-np.float64(np.asarray(plif_w)))))
    a_val = 1.0 - d
    MW = _prep_mw(conv_w, conv_b, bn_gamma, bn_beta, bn_mean, bn_var, d)

    x_aug = np.concatenate([x, np.ones((N, 1, T), np.float32)], axis=1).astype(
        np.float16
    )  # (N, 81, T)
    in_maps = []
    for i in range(N_CORES):
        xs = x_aug[i * NS : (i + 1) * NS]             # (128, 81, 500)
        xs_t = xs.transpose(1, 2, 0)                  # (81, 500, 128)
        xt = np.zeros((Cp1, NCOL, NS), np.float16)
        xt[:, 1:, :] = xs_t[:, T_ORDER, :]
        in_maps.append(
            {"x": np.ascontiguousarray(xt.reshape(Cp1, NCOL * NS)), "mw": MW}
        )
    return in_maps, a_val


def finish_output(results, fc_w, fc_b):
    """Host-side: vp tiles -> spikes -> spike counts -> features -> linear."""
    vp = np.concatenate([r["sp"] for r in results], axis=0)  # (N, B*FD) f16
    s = (vp >= np.float16(V_TH)).reshape(N, B, NB, Cp1)
    feat = s.sum(axis=(1, 2), dtype=np.float32) / np.float32(T)  # (N, 81)
    out = feat @ np.asarray(fc_w, np.float32).T + np.asarray(fc_b, np.float32)
    return out.astype(np.float32)


def get_program(a_val, paranoid=False):
    key = (round(a_val, 12), paranoid)
    if key not in _PROGRAM_CACHE:
        _PROGRAM_CACHE[key] = _build_program(a_val, paranoid)
    return _PROGRAM_CACHE[key]


def kernel(x, conv_w, conv_b, bn_gamma, bn_beta, bn_mean, bn_var, plif_w, fc_w, fc_b):
    in_maps, a_val = prep_inputs(
        x, conv_w, conv_b, bn_gamma, bn_beta, bn_mean, bn_var, plif_w
    )
    nc = get_program(a_val)
    res = run_bass_kernel_spmd(nc, in_maps, list(range(N_CORES)))
    return finish_output(res.results, fc_w, fc_b)
